# revision 14
# baseline (speedup 1.0000x reference)
"""MLA-v2 (multi-head latent attention) forward pass on 8 Trainium2 NeuronCores.

Sharding: core c -> (batch b = c // 4, head-group g = c % 4, 4 heads each).
Data parallel over batch; tensor parallel over heads (W_Q / W_up_K / W_up_V
column-sharded, W_O row-sharded).  The compressed latent c_kv is computed
replicated per core.  Each core emits one bf16 (S, D) partial; the host sums
the 4 partials per batch (the unshard step for row-parallel W_O).

Layout highlights (v2):
  * All activations/weights in bf16 (fp32 PSUM accumulation); Q/K stored as
    fp8e4m3 so QK^T runs in DoubleRow mode (0.5 PE cycles/row).
  * Q^T/K^T layout [128 part = 4 heads x 32 pair-idx, 2*S free]: even dh
    components in cols [0,S), odd in [S,2S).  RoPE becomes pure free-dim
    elementwise ops (no partition swaps); QK^T contracts (32 part x 2
    subtiles) per head via DoubleRow.
  * PV in (q, d) orientation: lhsT = probs tile, rhs = V-natural augmented
    with a ones column => full-rate PE + per-partition softmax denominators.
  * attn (q, d) -> (d, q) via DMA XBAR transpose (no PE/vector cost).
  * Front gemms (Q, c_kv, K, V) pipelined per 512-col slab with attention
    chunks (256 q) and the W_O gemm, to overlap PE work with the
    activation-engine exp stream.
"""

import numpy as np
import ml_dtypes

import concourse.bass as bass
import concourse.bacc as bacc
import concourse.mybir as mybir
import concourse.tile as tile
from concourse.bass_utils import run_bass_kernel_spmd

F32 = mybir.dt.float32
BF16 = mybir.dt.bfloat16
FP8 = mybir.dt.float8e4
DR = mybir.MatmulPerfMode.DoubleRow
EXP = mybir.ActivationFunctionType.Exp

B = 2
S = 2048
D = 1024
H = 16
DH = 64
DC = 256
HPC = 4          # heads per core
GD = HPC * DH    # per-core sharded model dim (256)
N_CORES = 8
NKT = D // 128   # k-tiles over D (8)
NCT = DC // 128  # k-tiles over DC (2)
NST = S // 128   # seq tiles (16)
NQC = S // 512   # 512-wide front slabs (4)
NCH = S // 256   # 256-wide attention chunks (8)
SCALE = 1.0 / float(np.sqrt(DH))


def _build_nc():
    nc = bacc.Bacc("TRN2", target_bir_lowering=False, debug=False,
                   num_devices=N_CORES)

    xt_d = nc.dram_tensor("xt", [D, S], BF16, kind="ExternalInput").ap()
    wq_d = nc.dram_tensor("wq", [D, 256], BF16, kind="ExternalInput").ap()
    wdkv_d = nc.dram_tensor("wdkv", [D, DC], BF16, kind="ExternalInput").ap()
    wupk_d = nc.dram_tensor("wupk", [DC, 256], BF16, kind="ExternalInput").ap()
    wupv_d = nc.dram_tensor("wupv", [DC, GD], BF16, kind="ExternalInput").ap()
    wo_d = nc.dram_tensor("wo", [GD, D], BF16, kind="ExternalInput").ap()
    ctab_d = nc.dram_tensor("ctab", [128, 2 * S], BF16, kind="ExternalInput").ap()
    stab_d = nc.dram_tensor("stab", [128, 2 * S], BF16, kind="ExternalInput").ap()
    mask_d = nc.dram_tensor("maskt", [128, 128], BF16, kind="ExternalInput").ap()
    out_d = nc.dram_tensor("out0", [S, D], BF16, kind="ExternalOutput").ap()

    with tile.TileContext(nc) as tc:
        with tc.tile_pool(name="sb", bufs=1) as sb, \
             tc.tile_pool(name="sbt", bufs=1) as sbt, \
             tc.tile_pool(name="psS", bufs=2, space="PSUM") as psS, \
             tc.tile_pool(name="psP", bufs=2, space="PSUM") as psP, \
             tc.tile_pool(name="psA", bufs=2, space="PSUM") as psA:

            # ---- persistent SBUF tensors ----
            xt = sb.tile([128, NKT * S], BF16, tag="xt")
            wq = sb.tile([128, NKT * 256], BF16, tag="wq")
            wdkv = sb.tile([128, NKT * DC], BF16, tag="wdkv")
            wupk = sb.tile([128, NCT * 256], BF16, tag="wupk")
            wupv = sb.tile([128, NCT * GD], BF16, tag="wupv")
            wo = sb.tile([128, NCT * D], BF16, tag="wo")
            ctab = sb.tile([128, 2 * S], BF16, tag="ctab")
            stab = sb.tile([128, 2 * S], BF16, tag="stab")
            maskt = sb.tile([128, 128], BF16, tag="maskt")
            qtmp = sb.tile([128, 2 * S], BF16, tag="qtmp")
            ktmp = sb.tile([128, 2 * S], BF16, tag="ktmp")
            qf8 = sb.tile([128, 2 * S], FP8, tag="qf8")
            kf8 = sb.tile([128, 2 * S], FP8, tag="kf8")
            # DoubleRow matmuls crash when consecutive instructions use
            # different PE row tile positions, so every head's Q/K operand is
            # staged at partition base 0: [32 pair-idx, head * (2, S)]
            qf8w = sb.tile([32, HPC * 2 * S], FP8, tag="qf8w")
            kf8w = sb.tile([32, HPC * 2 * S], FP8, tag="kf8w")
            cpair = sb.tile([128, NCT * S], BF16, tag="cpair")
            vaug = sb.tile([128, NST * 260], BF16, tag="vaug")
            attn_T = sb.tile([128, NCT * S], BF16, tag="attn_T")
            recips = sb.tile([128, NST * HPC], F32, tag="recips")

            qf8_r = qf8[:].rearrange("p (t s) -> p t s", t=2)
            kf8_r = kf8[:].rearrange("p (t s) -> p t s", t=2)
            qf8w_r = qf8w[:].rearrange("p (h t s) -> p h t s", h=HPC, t=2)
            kf8w_r = kf8w[:].rearrange("p (h t s) -> p h t s", h=HPC, t=2)

            def qk_ops(h):
                return kf8w_r[:, h], qf8w_r[:, h]

            def stage_qk(w_r, f_r, qc):
                for h in range(HPC):
                    nc.sync.dma_start(
                        w_r[:, h, :, qc * 512:(qc + 1) * 512],
                        f_r[32 * h:32 * h + 32, :, qc * 512:(qc + 1) * 512])

            # ---- input DMAs (order = DMA_ENGINES service order) ----
            nc.sync.dma_start(
                wq[:].rearrange("p (t n) -> p t n", t=NKT),
                wq_d.rearrange("(t p) n -> p t n", p=128))

            def dma_x(qc, t0=0, t1=NKT):
                nc.sync.dma_start(
                    xt[:].rearrange("p (t s) -> p t s", t=NKT)[
                        :, t0:t1, qc * 512:(qc + 1) * 512],
                    xt_d.rearrange("(t p) s -> p t s", p=128)[
                        :, t0:t1, qc * 512:(qc + 1) * 512])

            dma_x(0, 0, 4)
            dma_x(0, 4, 8)
            nc.sync.dma_start(
                wdkv[:].rearrange("p (t n) -> p t n", t=NKT),
                wdkv_d.rearrange("(t p) n -> p t n", p=128))
            nc.sync.dma_start(ctab[:], ctab_d)
            nc.sync.dma_start(stab[:], stab_d)
            nc.sync.dma_start(
                wupk[:].rearrange("p (t n) -> p t n", t=NCT),
                wupk_d.rearrange("(t p) n -> p t n", p=128))
            nc.sync.dma_start(
                wupv[:].rearrange("p (t n) -> p t n", t=NCT),
                wupv_d.rearrange("(t p) n -> p t n", p=128))
            dma_x(1)
            nc.sync.dma_start(maskt[:], mask_d)
            nc.sync.dma_start(
                wo[:].rearrange("p (t n) -> p t n", t=NCT),
                wo_d.rearrange("(t p) n -> p t n", p=128))
            dma_x(2)
            dma_x(3)

            # ones columns of V-augmented tiles (denominator accumulators)
            nc.any.memset(
                vaug[:].rearrange("p (q u) -> p q u", u=65)[:, :, 64:65], 1.0)

            def front_chain(dst, dst_off, w, w_stride, w_off, nkt,
                            rhs, rhs_stride, qc, name):
                acc = psA.tile([128, 512], F32, tag="fa", name=name)
                for kt in range(nkt):
                    nc.tensor.matmul(
                        acc[:],
                        w[:, kt * w_stride + w_off:kt * w_stride + w_off + 128],
                        rhs[:, kt * rhs_stride + qc * 512:
                            kt * rhs_stride + (qc + 1) * 512],
                        start=(kt == 0), stop=(kt == nkt - 1),
                    )
                nc.vector.tensor_copy(dst[:, dst_off:dst_off + 512], acc[:])

            def rope_piece(src, dst, qc, nm):
                # src bf16 [128, 2S] (evens | odds); dst fp8 same layout
                src3 = src[:].rearrange("p (t s) -> p t s", t=2)[
                    :, :, qc * 512:(qc + 1) * 512]
                dst3 = dst[:].rearrange("p (t s) -> p t s", t=2)[
                    :, :, qc * 512:(qc + 1) * 512]
                c3 = ctab[:].rearrange("p (t s) -> p t s", t=2)[
                    :, :, qc * 512:(qc + 1) * 512]
                s3 = stab[:].rearrange("p (t s) -> p t s", t=2)[
                    :, :, qc * 512:(qc + 1) * 512]
                y = sbt.tile([128, 1024], BF16, tag="ry", bufs=2,
                             name=f"ry{nm}")
                y3 = y[:].rearrange("p (t s) -> p t s", t=2, s=512)
                nc.vector.tensor_copy(y3[:, 0, :], src3[:, 1, :])
                nc.vector.tensor_copy(y3[:, 1, :], src3[:, 0, :])
                nc.vector.tensor_mul(src3, src3, c3)
                nc.vector.tensor_mul(y3, y3, s3)
                nc.gpsimd.tensor_add(dst3, src3, y3)

            def wo_gemm(qg):
                ost = sbt.tile([128, 1024], BF16, tag="ost", bufs=2,
                               name=f"ost{qg}")
                for nch in range(2):
                    accw = psA.tile([128, 512], F32, tag="fa",
                                    name=f"wo{qg}_{nch}")
                    for dblk in range(2):
                        nc.tensor.matmul(
                            accw[:],
                            attn_T[:, dblk * S + qg * 128:
                                   dblk * S + (qg + 1) * 128],
                            wo[:, dblk * D + nch * 512:
                               dblk * D + (nch + 1) * 512],
                            start=(dblk == 0), stop=(dblk == 1),
                        )
                    nc.vector.tensor_copy(ost[:, nch * 512:(nch + 1) * 512],
                                          accw[:])
                nc.sync.dma_start(out_d[qg * 128:(qg + 1) * 128, :], ost[:])

            pending_wo = []

            def attn_chunk(c):
                npair = c + 1
                pts = []
                for p_i in range(npair):
                    for hp in range(2):
                        st_t = psS.tile([128, 1024], F32, tag="st",
                                        name=f"st{c}_{p_i}_{hp}")
                        pt = sbt.tile([128, 1024], BF16, tag="pt", bufs=16,
                                      name=f"pt{c}_{p_i}_{hp}")
                        for t in range(2):
                            kt = 2 * p_i + t
                            lo = max(0, 128 * (kt - 2 * c))
                            for hl in range(2):
                                h = 2 * hp + hl
                                k_op, q_op = qk_ops(h)
                                nc.tensor.matmul(
                                    st_t[:, t * 512 + hl * 256 + lo:
                                         t * 512 + (hl + 1) * 256],
                                    k_op[:, :, kt * 128:(kt + 1) * 128],
                                    q_op[:, :, c * 256 + lo:(c + 1) * 256],
                                    start=(hl == 0), stop=(hl == 1),
                                    perf_mode=DR, skip_group_check=True,
                                )
                        if p_i < c:
                            nc.scalar.activation(pt[:], st_t[:], EXP,
                                                 scale=SCALE)
                        else:
                            # diagonal pair: exp only the causal region
                            nc.scalar.activation(pt[:, 0:512],
                                                 st_t[:, 0:512], EXP,
                                                 scale=SCALE)
                            in3 = st_t[:, 512:1024].rearrange(
                                "p (hl q) -> p hl q", q=256)[:, :, 128:256]
                            out3 = pt[:, 512:1024].rearrange(
                                "p (hl q) -> p hl q", q=256)[:, :, 128:256]
                            nc.scalar.activation(out3, in3, EXP, scale=SCALE)
                            for hl in range(2):
                                off = hl * 256
                                nc.vector.tensor_mul(pt[:, off:off + 128],
                                                    pt[:, off:off + 128],
                                                    maskt[:])
                                off = 512 + hl * 256 + 128
                                nc.vector.tensor_mul(pt[:, off:off + 128],
                                                    pt[:, off:off + 128],
                                                    maskt[:])
                        pts.append(pt)

                for qsl in range(2):
                    qg = 2 * c + qsl
                    acc = psP.tile([128, 512], F32, tag="pv",
                                   name=f"pv{qg}")
                    for p_i in range(npair):
                        for t in range(2):
                            kt = 2 * p_i + t
                            if kt > qg:
                                continue
                            for hp in range(2):
                                pt = pts[p_i * 2 + hp]
                                for hl in range(2):
                                    h = 2 * hp + hl
                                    nc.tensor.matmul(
                                        acc[:, h * 65:(h + 1) * 65],
                                        pt[:, t * 512 + hl * 256 + qsl * 128:
                                           t * 512 + hl * 256 + (qsl + 1) * 128],
                                        vaug[:, kt * 260 + h * 65:
                                             kt * 260 + (h + 1) * 65],
                                        start=(kt == 0 and h == 0),
                                        stop=(kt == qg and h == 3),
                                        skip_group_check=True,
                                    )
                    # softmax denominators -> reciprocals
                    nc.vector.reciprocal(
                        recips[:, qg * 4:qg * 4 + 4].rearrange(
                            "p (h u) -> p h u", u=1),
                        acc[:, 64:64 + 4 * 65].rearrange(
                            "p (h u) -> p h u", u=65)[:, :, 0:1])
                    anat = sbt.tile([128, 256], BF16, tag="anat", bufs=2,
                                    name=f"anat{qg}")
                    for h in range(HPC):
                        nc.vector.tensor_scalar(
                            anat[:, h * 64:(h + 1) * 64],
                            acc[:, h * 65:h * 65 + 64],
                            recips[:, qg * 4 + h:qg * 4 + h + 1], None,
                            mybir.AluOpType.mult)
                    nc.sync.dma_start_transpose(
                        attn_T[:].rearrange("p (d s) -> p d s", d=2)[
                            :, :, qg * 128:(qg + 1) * 128],
                        anat[:])
                    pending_wo.append(qg)

            # ---- pipelined front + attention + WO ----
            for qc in range(NQC):
                for eo in range(2):
                    front_chain(qtmp, eo * S + qc * 512, wq, 256, eo * 128,
                                NKT, xt, S, qc, f"q{eo}_{qc}")
                for ct in range(2):
                    front_chain(cpair, ct * S + qc * 512, wdkv, DC, ct * 128,
                                NKT, xt, S, qc, f"c{ct}_{qc}")
                rope_piece(qtmp, qf8, qc, f"q{qc}")
                stage_qk(qf8w_r, qf8_r, qc)
                for eo in range(2):
                    front_chain(ktmp, eo * S + qc * 512, wupk, 256, eo * 128,
                                NCT, cpair, S, qc, f"k{eo}_{qc}")
                rope_piece(ktmp, kf8, qc, f"k{qc}")
                stage_qk(kf8w_r, kf8_r, qc)
                for st in range(4 * qc, 4 * qc + 4):
                    accv = psA.tile([128, 512], F32, tag="fa",
                                    name=f"v{st}")
                    for ct in range(2):
                        nc.tensor.matmul(
                            accv[:, 0:GD],
                            cpair[:, ct * S + st * 128:ct * S + (st + 1) * 128],
                            wupv[:, ct * GD:(ct + 1) * GD],
                            start=(ct == 0), stop=(ct == 1),
                        )
                    nc.vector.tensor_copy(
                        vaug[:, st * 260:(st + 1) * 260].rearrange(
                            "p (h u) -> p h u", u=65)[:, :, 0:64],
                        accv[:, 0:GD].rearrange("p (h u) -> p h u", u=64))

                for c in (2 * qc, 2 * qc + 1):
                    # trail the W_O gemm by one chunk so the XBAR transpose
                    # is never on the PE critical path
                    while len(pending_wo) > 2:
                        wo_gemm(pending_wo.pop(0))
                    attn_chunk(c)
            while pending_wo:
                wo_gemm(pending_wo.pop(0))

    nc.compile()
    return nc


_NC_CACHE = []


def _get_nc():
    if not _NC_CACHE:
        _NC_CACHE.append(_build_nc())
    return _NC_CACHE[0]


def _host_tables():
    theta = 10000.0 ** (-np.arange(0, DH, 2, dtype=np.float64) / DH)  # (32,)
    pos = np.arange(S, dtype=np.float64)
    ang = np.outer(theta, pos)  # (32, S)
    cos = np.cos(ang)
    sin = np.sin(ang)
    cos4 = np.tile(cos, (4, 1))  # (128, S) rows = (head, pair-idx)
    sin4 = np.tile(sin, (4, 1))
    ctab = np.concatenate([cos4, cos4], axis=1).astype(ml_dtypes.bfloat16)
    stab = np.concatenate([-sin4, sin4], axis=1).astype(ml_dtypes.bfloat16)
    mask = (np.arange(128)[None, :] >= np.arange(128)[:, None])
    mask = mask.astype(ml_dtypes.bfloat16)  # (128,128): 1 where q >= k
    return ctab, stab, mask


def _make_in_maps(X, W_Q, W_down_kv, W_up_K, W_up_V, W_O):
    X = np.asarray(X, np.float32)
    W_Q = np.asarray(W_Q, np.float32)
    W_down_kv = np.asarray(W_down_kv, np.float32)
    W_up_K = np.asarray(W_up_K, np.float32)
    W_up_V = np.asarray(W_up_V, np.float32)
    W_O = np.asarray(W_O, np.float32)

    ctab, stab, mask = _host_tables()
    bf = ml_dtypes.bfloat16

    in_maps = []
    for c in range(N_CORES):
        b, g = c // 4, c % 4
        cols_e = np.concatenate(
            [(g * HPC + h) * DH + np.arange(0, DH, 2) for h in range(HPC)])
        cols_o = cols_e + 1
        cols = np.concatenate([cols_e, cols_o])  # (256,): [4h evens | 4h odds]
        vcols = slice(g * GD, (g + 1) * GD)
        in_maps.append({
            "xt": np.ascontiguousarray(X[b].T).astype(bf),
            "wq": np.ascontiguousarray(W_Q[:, cols]).astype(bf),
            "wdkv": W_down_kv.astype(bf),
            "wupk": np.ascontiguousarray(W_up_K[:, cols]).astype(bf),
            "wupv": np.ascontiguousarray(W_up_V[:, vcols]).astype(bf),
            "wo": np.ascontiguousarray(W_O[vcols, :]).astype(bf),
            "ctab": ctab, "stab": stab, "maskt": mask,
        })

    return in_maps


def _gather(res):
    out = np.zeros((B, S, D), np.float32)
    for c in range(N_CORES):
        out[c // 4] += res.results[c]["out0"].astype(np.float32)
    return out


def kernel(X, W_Q, W_down_kv, W_up_K, W_up_V, W_O):
    in_maps = _make_in_maps(X, W_Q, W_down_kv, W_up_K, W_up_V, W_O)
    nc = _get_nc()
    res = run_bass_kernel_spmd(nc, in_maps, core_ids=list(range(N_CORES)))
    return _gather(res)
